# revision 12
# baseline (speedup 1.0000x reference)
"""OIM loss with circular queue — Trainium2 Bass kernel (8 NeuronCores).

Strategy
--------
The output is a scalar:  loss = mean_b [ logsumexp_{q in good}(30*cos(x_b, e_q))
                                         - 30*cos(x_b, e_{xe_b}) ]
where e is the circular queue after the (sequential, data-dependent) update.

The queue update only *moves integer labels around* plus writes U=256
normalized per-pid mean embeddings into a contiguous window of slots.  All the
integer bookkeeping (which slots are invalidated, which slot each batch row
targets) is done on the host; every FLOP-heavy part runs on the 8 cores:

  - per-pid masked means  (one-hot mask matmul,  [U,B]x[B,D])
  - row normalization of inputs and means
  - the big logits matmul [B,Q//8,D] per core (float32r, full PE rate)
    fused with exp (ACT: exp(30*s - M), M a safe upper bound of the row max)
    and the masked row-sum (DVE tensor_tensor_reduce with the `good` mask)
  - target cosines via a small [B,U] matmul + one-hot gather

Sharding: emb_cq is sharded over Q (2048 rows/core, tensor-parallel); the
batch-side preprocessing is replicated (it is ~2% of the FLOPs).  Each core
returns partial sums of exp(logit - M) over its Q-shard plus the target
cosines; the host adds the 8 partials (the "allreduce"), takes log and means.
"""

import os
import sys

import numpy as np

for _p in ("/opt/trn_rl_repo", "/root/.axon_site/_ro/trn_rl_repo"):
    if os.path.isdir(_p) and _p not in sys.path:
        sys.path.insert(0, _p)

B, D, Q, U = 4096, 512, 16384, 256
N_CORES = 8
QS = Q // N_CORES          # queue rows per core
OIM_SCALAR = 30.0
IGNORE = -1
MT = B // 128              # 32 b-tiles
QT = QS // 128             # 16 q-tiles per core
KD = D // 128              # 4 contraction chunks
NQ = QS // 512             # 4 matmul n-chunks per core
UT = U // 128              # 2 u-tiles

_PROG_CACHE = {}


def _build_program(M: float):
    """Emit + schedule + compile the (SPMD, identical on all cores) program."""
    import concourse.bacc as bacc
    import concourse.bass as bass
    import concourse.tile as tile
    from concourse import mybir
    from concourse.masks import make_identity

    f32 = mybir.dt.float32
    f32r = mybir.dt.float32r
    AF = mybir.ActivationFunctionType
    OP = mybir.AluOpType

    nc = bacc.Bacc("TRN2", target_bir_lowering=False, debug=False,
                   num_devices=N_CORES)

    x_d = nc.dram_tensor("x", [B, D], f32, kind="ExternalInput").ap()
    emb_d = nc.dram_tensor("emb", [QS, D], f32, kind="ExternalInput").ap()
    labf_d = nc.dram_tensor("labf", [128, MT], f32, kind="ExternalInput").ap()
    uniqf_d = nc.dram_tensor("uniqf", [128, U], f32, kind="ExternalInput").ap()
    cnts_d = nc.dram_tensor("cnts", [128, UT], f32, kind="ExternalInput").ap()
    widx_d = nc.dram_tensor("widx", [128, MT], f32, kind="ExternalInput").ap()
    iota_d = nc.dram_tensor("iota", [128, U], f32, kind="ExternalInput").ap()
    gkeep_d = nc.dram_tensor("gkeep", [128, QS], f32, kind="ExternalInput").ap()
    wkeep_d = nc.dram_tensor("wkeep", [128, QT], f32, kind="ExternalInput").ap()
    oht_d = nc.dram_tensor("oht", [128, UT, QS], f32, kind="ExternalInput").ap()
    sume_d = nc.dram_tensor("sume", [128, MT], f32, kind="ExternalOutput").ap()
    tco_d = nc.dram_tensor("tco", [128, MT], f32, kind="ExternalOutput").ap()

    with tile.TileContext(nc) as tc:
        with (
            tc.tile_pool(name="singles", bufs=1) as singles,
            tc.tile_pool(name="work", bufs=3) as work,
            tc.tile_pool(name="small", bufs=6) as small,
            tc.tile_pool(name="psum_t", bufs=2, space="PSUM") as psum_t,
        ):
            # ---------------- constants / small inputs ----------------
            ident = singles.tile([128, 128], f32)
            make_identity(nc, ident)

            labs = singles.tile([128, MT], f32)
            nc.sync.dma_start(out=labs, in_=labf_d)
            widx = singles.tile([128, MT], f32)
            nc.sync.dma_start(out=widx, in_=widx_d)
            wkp = singles.tile([128, QT], f32)
            nc.sync.dma_start(out=wkp, in_=wkeep_d)
            cnts = singles.tile([128, UT], f32)
            nc.sync.dma_start(out=cnts, in_=cnts_d)
            uniqb = singles.tile([128, U], f32)
            nc.sync.dma_start(out=uniqb, in_=uniqf_d)
            iotab = singles.tile([128, U], f32)
            nc.sync.dma_start(out=iotab, in_=iota_d)
            keepg = singles.tile([128, QS], f32)
            nc.sync.dma_start(out=keepg, in_=gkeep_d)
            oht = singles.tile([128, UT, QS], f32r)
            nc.sync.dma_start(out=oht, in_=oht_d.bitcast(f32r))

            rcnt = singles.tile([128, UT], f32)
            nc.vector.reciprocal(rcnt, cnts)
            biasM = singles.tile([128, 1], f32)
            nc.vector.memset(biasM, -M)

            # resident big tensors
            xn_all = singles.tile([128, MT, D], f32)     # normalized inputs (b-major)
            embT = singles.tile([128, KD, QS], f32r)     # blended emb, d-major
            uembT = singles.tile([128, KD, U], f32r)     # uniq means, d-major
            uemb_n = singles.tile([128, UT, D], f32r)    # uniq means, u-major
            ssb = singles.tile([128, MT], f32)           # sum-exp out collector
            tsb = singles.tile([128, MT], f32)           # target-cos out collector

            # ---------------- phase 1+2: masked means + normalize ----------
            with tc.tile_pool(name="psum_u", bufs=1, space="PSUM") as psum_u:
                ps_u = [psum_u.tile([128, D], f32, tag=f"uniq{mu}",
                                    name=f"ps_u{mu}") for mu in range(UT)]
                for i in range(MT):
                    x_raw = work.tile([128, D], f32r, tag="x_raw")
                    nc.sync.dma_start(out=x_raw,
                                      in_=x_d[i * 128:(i + 1) * 128, :].bitcast(f32r))
                    x_f = x_raw.bitcast(f32)

                    # mask[b, u] = (uniq[u] == labels[b])
                    mt_ = work.tile([128, U], f32r, tag="maskr")
                    nc.vector.tensor_scalar(out=mt_, in0=uniqb,
                                            scalar1=labs[:, i:i + 1], scalar2=None,
                                            op0=OP.is_equal)
                    for mu in range(UT):
                        nc.tensor.matmul(ps_u[mu],
                                         mt_[:, mu * 128:(mu + 1) * 128],
                                         x_raw, start=(i == 0),
                                         stop=(i == MT - 1))

                    # row-normalize x
                    sq = work.tile([128, D], f32, tag="sq")
                    ssq = small.tile([128, 1], f32, tag="ssq")
                    nc.vector.scalar_tensor_tensor(out=sq, in0=x_f, scalar=1.0,
                                                   in1=x_f, op0=OP.mult,
                                                   op1=OP.mult, accum_out=ssq)
                    nrm = small.tile([128, 1], f32, tag="nrm")
                    nc.scalar.activation(out=nrm, in_=ssq, func=AF.Sqrt)
                    nc.vector.tensor_scalar_max(out=nrm, in0=nrm, scalar1=1e-12)
                    rin = small.tile([128, 1], f32, tag="rin")
                    nc.vector.reciprocal(rin, nrm)
                    nc.vector.tensor_scalar_mul(out=xn_all[:, i, :], in0=x_f,
                                                scalar1=rin)

                # finish uniq means: mean, normalize, transpose to d-major
                for mu in range(UT):
                    ue = uemb_n[:, mu, :]
                    nc.vector.tensor_scalar_mul(out=ue, in0=ps_u[mu],
                                                scalar1=rcnt[:, mu:mu + 1])
                    sq2 = work.tile([128, D], f32, tag="sq")
                    ssq2 = small.tile([128, 1], f32, tag="ssq")
                    ue_f = ue.bitcast(f32)
                    nc.vector.scalar_tensor_tensor(out=sq2, in0=ue_f, scalar=1.0,
                                                   in1=ue_f, op0=OP.mult,
                                                   op1=OP.mult, accum_out=ssq2)
                    nrm2 = small.tile([128, 1], f32, tag="nrm")
                    nc.scalar.activation(out=nrm2, in_=ssq2, func=AF.Sqrt)
                    nc.vector.tensor_scalar_max(out=nrm2, in0=nrm2, scalar1=1e-12)
                    rin2 = small.tile([128, 1], f32, tag="rin")
                    nc.vector.reciprocal(rin2, nrm2)
                    nc.vector.tensor_scalar_mul(out=ue, in0=ue_f, scalar1=rin2)
                    for kd in range(KD):
                        pst = psum_t.tile([128, 128], f32, tag="pst")
                        nc.tensor.transpose(
                            pst,
                            uemb_n[:, mu, kd * 128:(kd + 1) * 128].bitcast(f32),
                            ident)
                        nc.scalar.copy(out=uembT[:, kd, mu * 128:(mu + 1) * 128],
                                       in_=pst)

            # ---------------- phase 3: blend queue window + transpose ------
            with tc.tile_pool(name="psum_b", bufs=2, space="PSUM") as psum_b:
                for t in range(QT):
                    e_raw = work.tile([128, D], f32, tag="e_raw")
                    nc.sync.dma_start(out=e_raw,
                                      in_=emb_d[t * 128:(t + 1) * 128, :])
                    eff = work.tile([128, D], f32, tag="eff")
                    # zero the window rows ...
                    nc.vector.tensor_scalar_mul(out=eff, in0=e_raw,
                                                scalar1=wkp[:, t:t + 1])
                    # ... and add one-hot @ uniq_means
                    psb = psum_b.tile([128, D], f32, tag="psb")
                    for ku in range(UT):
                        nc.tensor.matmul(psb,
                                         oht[:, ku, t * 128:(t + 1) * 128],
                                         uemb_n[:, ku, :],
                                         start=(ku == 0), stop=(ku == UT - 1))
                    nc.vector.tensor_add(out=eff, in0=eff, in1=psb)
                    for kd in range(KD):
                        pst = psum_t.tile([128, 128], f32, tag="pst")
                        nc.tensor.transpose(pst, eff[:, kd * 128:(kd + 1) * 128],
                                            ident)
                        nc.scalar.copy(out=embT[:, kd, t * 128:(t + 1) * 128],
                                       in_=pst)

            # ---------------- phase 4: logits + fused LSE ----------------
            with (
                tc.tile_pool(name="psum_s", bufs=2, space="PSUM") as psum_s,
                tc.tile_pool(name="psum_m", bufs=3, space="PSUM") as psum_m,
            ):
                for m in range(MT):
                    tl = work.tile([128, D], f32r, tag="lhsT")
                    for kd in range(KD):
                        pst = psum_t.tile([128, 128], f32, tag="pst")
                        nc.tensor.transpose(
                            pst, xn_all[:, m, kd * 128:(kd + 1) * 128], ident)
                        nc.scalar.copy(out=tl[:, kd * 128:(kd + 1) * 128], in_=pst)

                    # target cosines: S2[b, u] then one-hot gather along u
                    pss = psum_s.tile([128, U], f32, tag="pss")
                    for kd in range(KD):
                        nc.tensor.matmul(pss, tl[:, kd * 128:(kd + 1) * 128],
                                         uembT[:, kd, :],
                                         start=(kd == 0), stop=(kd == KD - 1))
                    scr_u = work.tile([128, U], f32, tag="mask")
                    nc.vector.scalar_tensor_tensor(out=scr_u, in0=iotab,
                                                   scalar=widx[:, m:m + 1],
                                                   in1=pss,
                                                   op0=OP.is_equal, op1=OP.mult,
                                                   accum_out=tsb[:, m:m + 1])

                    # big matmul over this core's Q-shard, fused exp+masked sum
                    acc4 = small.tile([128, NQ], f32, tag="acc4")
                    for n in range(NQ):
                        psm = psum_m.tile([128, 512], f32, tag="psm")
                        for kd in range(KD):
                            nc.tensor.matmul(
                                psm, tl[:, kd * 128:(kd + 1) * 128],
                                embT[:, kd, n * 512:(n + 1) * 512],
                                start=(kd == 0), stop=(kd == KD - 1))
                        expt = work.tile([128, 512], f32, tag="expt")
                        nc.scalar.activation(out=expt, in_=psm, func=AF.Exp,
                                             bias=biasM, scale=OIM_SCALAR)
                        scr = work.tile([128, 512], f32, tag="scr")
                        nc.vector.scalar_tensor_tensor(
                            out=scr, in0=expt, scalar=1.0,
                            in1=keepg[:, n * 512:(n + 1) * 512],
                            op0=OP.mult, op1=OP.mult,
                            accum_out=acc4[:, n:n + 1])
                    nc.vector.reduce_sum(out=ssb[:, m:m + 1], in_=acc4,
                                         axis=mybir.AxisListType.X)

            nc.sync.dma_start(out=sume_d, in_=ssb)
            nc.sync.dma_start(out=tco_d, in_=tsb)

    nc.compile()
    return nc


def _host_bookkeeping(labels, label_cq, header_cq):
    """Mirror the reference's integer-only queue-update semantics."""
    labels = np.asarray(labels).astype(np.int64)
    lab = np.asarray(label_cq).astype(np.int64).copy()
    h0 = int(np.asarray(header_cq))

    # jnp.unique(labels, size=U): sorted unique, padded with the minimum
    uq = np.unique(labels)
    if uq.size < U:
        uniq = np.concatenate([uq, np.full(U - uq.size, uq.min(), np.int64)])
    else:
        uniq = uq[:U]
    cnts = np.array([(labels == v).sum() for v in uniq], np.int64)

    emb_src = np.full(Q, -1, np.int64)   # >=0: row u of uniq means; -1: original
    h = h0 % Q
    for u in range(U):
        y = uniq[u]
        m = lab == y
        i = int(np.argmax(m)) if m.any() else 0
        inval = bool(m.any()) and (i != h)
        emb_src[h] = u
        lab[h] = y
        if inval:
            lab[i] = IGNORE
        h = (h + 1) % Q

    good = lab != IGNORE
    goodidx = np.flatnonzero(good)
    gl = lab[goodidx]
    vals, first = np.unique(gl, return_index=True)
    pos = np.searchsorted(vals, labels)
    assert np.all(vals[np.clip(pos, 0, vals.size - 1)] == labels), \
        "batch label missing from queue"
    xe = goodidx[first[pos]]
    return uniq, cnts, emb_src, good, xe


def _prepare(inputs, labels, emb_cq, label_cq, header_cq):
    """Host bookkeeping -> (M, per-core input maps, extra-target indices, xe)."""
    inputs = np.ascontiguousarray(np.asarray(inputs, np.float32))
    emb_cq = np.ascontiguousarray(np.asarray(emb_cq, np.float32))

    uniq, cnts, emb_src, good, xe = _host_bookkeeping(labels, label_cq, header_cq)

    # safe upper bound for any logit: 30 * max row norm (uniq means have norm 1)
    max_nrm = float(np.sqrt((emb_cq.astype(np.float64) ** 2).sum(axis=1).max()))
    M = OIM_SCALAR * max(1.0, max_nrm) * 1.0000001

    w_idx = emb_src[xe].astype(np.float64)        # -1 for non-window targets
    extra = np.flatnonzero(w_idx < 0)             # handled on host (rare/none)

    def pmajor(v, cols):
        return np.ascontiguousarray(
            np.asarray(v, np.float32).reshape(cols, 128).T)

    base = {
        "x": inputs,
        "labf": pmajor(np.asarray(labels, np.float64), MT),
        "uniqf": np.ascontiguousarray(
            np.broadcast_to(uniq.astype(np.float32), (128, U))),
        "cnts": pmajor(cnts, UT),
        "widx": pmajor(w_idx, MT),
        "iota": np.ascontiguousarray(
            np.broadcast_to(np.arange(U, dtype=np.float32), (128, U))),
    }
    in_maps = []
    for c in range(N_CORES):
        sl = slice(c * QS, (c + 1) * QS)
        src = emb_src[sl]
        ohtT = np.zeros((U, QS), np.float32)
        j = np.flatnonzero(src >= 0)
        ohtT[src[j], j] = 1.0
        in_maps.append({
            **base,
            "emb": np.ascontiguousarray(emb_cq[sl]),
            "gkeep": np.ascontiguousarray(
                np.broadcast_to(good[sl].astype(np.float32), (128, QS))),
            "wkeep": pmajor((src < 0).astype(np.float32), QT),
            "oht": np.ascontiguousarray(
                ohtT.reshape(UT, 128, QS).transpose(1, 0, 2)),
        })
    return M, in_maps, extra, xe


def _combine(res_list, M, extra, xe, inputs, emb_cq):
    """Unshard / combine per-core partials into the scalar loss."""
    S = np.zeros(B, np.float64)
    for r in res_list:
        S += r["sume"].astype(np.float64).T.reshape(B)
    t_cos = res_list[0]["tco"].astype(np.float64).T.reshape(B)

    if extra.size:  # targets pointing at original (non-window) queue rows
        xb = np.asarray(inputs, np.float64)[extra]
        xb /= np.maximum(np.linalg.norm(xb, axis=1, keepdims=True), 1e-12)
        eb = np.asarray(emb_cq, np.float64)[xe[extra]]
        t_cos[extra] = (xb * eb).sum(axis=1)

    loss = np.mean(M + np.log(S) - OIM_SCALAR * t_cos)
    return np.float32(loss)


def kernel(inputs, labels, emb_cq, label_cq, age_cq, header_cq):
    from concourse.bass_utils import run_bass_kernel_spmd

    M, in_maps, extra, xe = _prepare(inputs, labels, emb_cq, label_cq, header_cq)

    key = round(M, 9)
    if key not in _PROG_CACHE:
        _PROG_CACHE[key] = _build_program(M)
    nc = _PROG_CACHE[key]

    res = run_bass_kernel_spmd(nc, in_maps, core_ids=list(range(N_CORES)))
    return _combine(res.results, M, extra, xe, inputs, emb_cq)
